# revision 26
# baseline (speedup 1.0000x reference)
"""Trainium2 Bass kernel for complex-valued multi-head attention with key masking.

Problem (hardcoded shapes): B=4, Nq=Nk=1024, R=256, NH=8, DK=DV=64.
  Q,K,V complex [B,N,R] (given as _real/_imag f32 pairs), complex weights
  WQ/WK/WV [512,256], WO [256,512], boolean key mask [B,Nk].
  out = complex MHA(Q,K,V) with softmax over |scores| restricted to valid keys.

Sharding: 8 cores = (batch b in 0..3) x (head-group hg in 0..1, 4 heads each).
Each core computes its batch's attention for its 4 heads plus the partial
output projection; the host sums the two head-group partials per batch.

Device-side layout: channels on partitions, sequence on the free dim, with
complex arithmetic folded into the matmuls by stacking real/imag along the
128-partition contraction dim (Qa=[Qp_r;Qp_i], Qb=[Qp_i;-Qp_r], Ka=[Kp_r;Kp_i]
give Sr/Si with one full-width matmul each).

Key structural idea vs a [q,k]-softmax design: scores are computed
TRANSPOSED, S^T[k,q] = Ka_blk^T.Qa per 128-key block, so the whole softmax
chain (|s|^2 via one fused two-source DVE op, sqrt, exp) runs in [k,q]
layout and the exp output E^T is consumed by the attention matmul straight
from SBUF -- no transpose DMA, no DRAM bounce.  The softmax denominator
den[h,q] = sum_k E^T is produced on the PE by a ones-column matmul
accumulated into a [4,1024] PSUM tile, moved to [q,4-heads] orientation by a
tiny identity matmul, reciprocal'd once, and the per-head 1/den is folded
into the output-projection reduction (out_q[q,r] = sum_h (att_h^T.WO_h)/den_h)
which runs in q-partition orientation so the scale is a per-partition scalar.
Masked keys are removed by host-side compaction (padded keys contribute
exp(0)=1 to den, subtracted via a host-provided count).
"""

import numpy as np
import ml_dtypes

B, NQ, NK, R = 4, 1024, 1024, 256
NH, DK, DV = 8, 64, 64
NCORES = 8
NHL = 4          # heads per core
F32MIN_PAD = 640  # minimum padded key count (keys padded to a multiple of 128)

_BF16 = ml_dtypes.bfloat16

# ----------------------------------------------------------------------------
# custom DVE op (registered at import into concourse's op table)
# ----------------------------------------------------------------------------
_OPS = {}


def _register_custom_ops():
    if _OPS:
        return
    import concourse.dve_ops as dom
    from concourse.dve_ops import DveOp
    from concourse.dve_spec import Spec, Src0, Src1, C0, sq, lower, _has_src1
    from concourse.dve_uop import DveOpSpec

    def make(name, spec):
        if name in dom._SUB_OPCODE_FOR_NAME:
            _OPS[name] = next(o for o in dom.OPS if o.name == name)
            return
        row = dom._CUSTOM_DVE_ROW_BASE + len(dom.OPS)
        assert row < 0x20, "custom DVE row overflow"
        shas = {}
        for ver in ("v3", "v4"):
            tmp = DveOpSpec(name=name, opcode=row, uops=lower(spec, ver=ver),
                            rd1_en=_has_src1(spec))
            shas[ver] = tmp.sha(ver)
        op = DveOp(name, spec, subdim=False, uops_sha=shas)
        dom.OPS.append(op)
        dom._SUB_OPCODE_FOR_NAME[name] = row
        dom.CUSTOM_DVE_SPECS[name] = spec
        _OPS[name] = op

    # t = (in0*s0)^2          (drains+squares one score tile from PSUM)
    make("CMHA_SQSC", Spec(
        body=sq(Src0 * C0),
        reference=lambda in0, in1, s0, s1, imm2: (in0.astype(np.float32) * s0) ** 2,
    ))
    # u = (in0*s0)^2 + in1    (second square + accumulate |s/8|^2; the DVE
    # can read at most one input from PSUM, so this pairs with CMHA_SQSC)
    make("CMHA_SQADD", Spec(
        body=sq(Src0 * C0) + Src1,
        reference=lambda in0, in1, s0, s1, imm2: (in0.astype(np.float32) * s0) ** 2
        + in1.astype(np.float32),
    ))


# ----------------------------------------------------------------------------
# device program
# ----------------------------------------------------------------------------
_BUILD_CACHE = {}


def _build(nkp):
    """Build + compile the SPMD device program for padded key count nkp."""
    if nkp in _BUILD_CACHE:
        return _BUILD_CACHE[nkp]
    _register_custom_ops()
    import concourse.bass as bass
    import concourse.bacc as bacc
    import concourse.mybir as mybir
    import concourse.tile as tile
    from contextlib import ExitStack

    F32 = mybir.dt.float32
    BF16 = mybir.dt.bfloat16
    AF = mybir.ActivationFunctionType
    assert nkp % 128 == 0
    KB = nkp // 128                  # 128-sized key blocks, all full

    nc = bacc.Bacc("TRN2", target_bir_lowering=False, debug=False,
                   num_devices=NCORES)

    qt = nc.dram_tensor("qt", [512, NQ], BF16, kind="ExternalInput").ap()
    kt = nc.dram_tensor("kt", [512, nkp], BF16, kind="ExternalInput").ap()
    vt = nc.dram_tensor("vt", [512, nkp], BF16, kind="ExternalInput").ap()
    wq = nc.dram_tensor("wq", [NHL, 512, 256], BF16, kind="ExternalInput").ap()
    wk = nc.dram_tensor("wk", [NHL, 512, 128], BF16, kind="ExternalInput").ap()
    wv = nc.dram_tensor("wv", [512, 512], BF16, kind="ExternalInput").ap()
    wo = nc.dram_tensor("wo", [NHL, 128, 512], BF16, kind="ExternalInput").ap()
    ones = nc.dram_tensor("ones", [128, 1], BF16, kind="ExternalInput").ap()
    npn = nc.dram_tensor("npn", [128, 1], F32, kind="ExternalInput").ap()
    outq = nc.dram_tensor("outq", [NQ, 512], F32, kind="ExternalOutput").ap()

    sqsc = _OPS["CMHA_SQSC"]
    sqadd = _OPS["CMHA_SQADD"]
    ADD = mybir.AluOpType.add
    MULT = mybir.AluOpType.mult

    with tile.TileContext(nc) as tc, ExitStack() as ctx:
        const = ctx.enter_context(tc.tile_pool(name="const", bufs=1))
        psum = ctx.enter_context(tc.tile_pool(name="psum", bufs=1, space="PSUM"))
        outp = ctx.enter_context(tc.tile_pool(name="outp", bufs=4))

        # ---- input loads: all on the hardware-DGE engines (sync/scalar);
        # gpsimd software descriptor-gen costs ~0.8us per DMA and delays
        # the first matmul.  Queue order matches consumption order.
        def load(shape, dtype, src, tag, eng):
            t = const.tile(shape, dtype, tag=tag, name=tag)
            eng.dma_start(t[:], src)
            return t

        qt_sb = [load([128, NQ], BF16, qt[c * 128:(c + 1) * 128, :], f"qt{c}",
                      nc.sync) for c in range(4)]
        # wq[h] is [512, 256] = 4 contraction chunks; load per-head in one DMA
        wq_t = []
        for h in range(NHL):
            t = const.tile([128, 1024], BF16, tag=f"wq{h}", name=f"wqt{h}")
            nc.sync.dma_start(
                t[:].rearrange("p (c n) -> p c n", c=4),
                wq[h].rearrange("(c p) n -> p c n", p=128))
            wq_t.append(t)
        wq_sb = [[wq_t[h][:, c * 256:(c + 1) * 256] for c in range(4)]
                 for h in range(NHL)]
        kt_sb = [load([128, nkp], BF16, kt[c * 128:(c + 1) * 128, :], f"kt{c}",
                      nc.scalar) for c in range(4)]
        wk_t = []
        for h in range(NHL):
            t = const.tile([128, 512], BF16, tag=f"wk{h}", name=f"wkt{h}")
            nc.scalar.dma_start(
                t[:].rearrange("p (c n) -> p c n", c=4),
                wk[h].rearrange("(c p) n -> p c n", p=128))
            wk_t.append(t)
        wk_sb = [[wk_t[h][:, c * 128:(c + 1) * 128] for c in range(4)]
                 for h in range(NHL)]
        vt_sb = [load([128, nkp], BF16, vt[c * 128:(c + 1) * 128, :], f"vt{c}",
                      nc.sync) for c in range(4)]
        wv_sb = [load([128, 512], BF16, wv[c * 128:(c + 1) * 128, :], f"wv{c}",
                      nc.scalar) for c in range(4)]
        wo_sb = [load([128, 512], BF16, wo[h], f"wo{h}", nc.scalar)
                 for h in range(NHL)]
        ones_sb = load([128, 1], BF16, ones[:], "ones", nc.sync)
        npn_sb = load([128, 1], F32, npn[:], "npn", nc.sync)

        VK = const.tile([128, 512 * KB], BF16, tag="vk", name="VK")

        # PSUM tags: 4x [128,512] (sr0/sr1/si0/si1 rings, reused by the
        # projections, the den transpose and the outproj partials), one
        # [128,1024] attn accumulator, one [4,1024] den accumulator = 8 banks.
        def ps512(tag):
            return psum.tile([128, 512], F32, tag=tag, name=tag)

        def mm(out_ap, lhsT, rhs, start=True, stop=True, skip=False):
            nc.tensor.matmul(out_ap, lhsT, rhs, start=start, stop=stop,
                             skip_group_check=skip)

        # ---- phase A: projections --------------------------------------
        # copies alternate vector/scalar (gpsimd cannot read PSUM on trn2)
        _cp_i = [0]

        def copy(dst, src):
            _cp_i[0] += 1
            if _cp_i[0] % 2:
                nc.vector.tensor_copy(dst, src)
            else:
                nc.scalar.copy(dst, src)

        Qa, Qb, Ka = [], [], []
        for h in range(NHL):
            qa = const.tile([128, NQ], BF16, tag=f"qa{h}", name=f"Qa{h}")
            qb = const.tile([128, NQ], BF16, tag=f"qb{h}", name=f"Qb{h}")
            for qc in range(2):
                pa = ps512("sr0" if qc == 0 else "sr1")
                pb = ps512("si0" if qc == 0 else "si1")
                for c in range(4):
                    mm(pa[:], wq_sb[h][c][:, 0:128],
                       qt_sb[c][:, qc * 512:(qc + 1) * 512], c == 0, c == 3)
                for c in range(4):
                    mm(pb[:], wq_sb[h][c][:, 128:256],
                       qt_sb[c][:, qc * 512:(qc + 1) * 512], c == 0, c == 3)
                copy(qa[:, qc * 512:(qc + 1) * 512], pa[:])
                copy(qb[:, qc * 512:(qc + 1) * 512], pb[:])
            Qa.append(qa)
            Qb.append(qb)

            ka = const.tile([128, nkp], BF16, tag=f"ka{h}", name=f"Ka{h}")
            for o in range(0, nkp, 512):
                w_ = min(512, nkp - o)
                pk = ps512("sr0" if (o // 512) % 2 == 0 else "sr1")
                for c in range(4):
                    mm(pk[0:128, 0:w_], wk_sb[h][c][:],
                       kt_sb[c][:, o:o + w_], c == 0, c == 3)
                copy(ka[:, o:o + w_], pk[0:128, 0:w_])
            Ka.append(ka)

        for kb in range(KB):
            pv = ps512("si0" if kb % 2 == 0 else "si1")
            for c in range(4):
                mm(pv[:], vt_sb[c][:, kb * 128:(kb + 1) * 128],
                   wv_sb[c][:], c == 0, c == 3)
            copy(VK[0:128, kb * 512:(kb + 1) * 512], pv[:])

        # ---- phase B: head-pair pipeline ---------------------------------
        # DVE can read only ONE PSUM input per op, so |s|^2 is a two-op
        # chain: t = (Sr/8)^2 (SQSC on DVE, or Square on ACT -- Square is
        # in every ACT table), u = (Si/8)^2 + t (SQADD, DVE).  Heads run in
        # pairs: {scores 01} {sqrt 01} {scores 23, exp 01 + attn/den 01}
        # {sqrt 23} {exp 23 + attn/den 23}, so the PE stays dense while the
        # ACT table only switches 4x and attention streams behind exp.
        u_t = [const.tile([128, KB * NQ], BF16, tag=f"u{h}", name=f"u{h}")
               for h in range(NHL)]
        ATT = [None] * NHL

        def emit_scores(h, act_frac):
            # act_frac of the first-squares go to ACT, rest to DVE
            u = u_t[h]
            nsq = 0
            for kb in range(KB):
                ka_sl = Ka[h][:, kb * 128:(kb + 1) * 128]
                t = const.tile([128, NQ], BF16, tag="tsq", bufs=3, name="tsq")
                for qc in range(2):
                    sr = ps512("sr0" if qc == 0 else "sr1")
                    si = ps512("si0" if qc == 0 else "si1")
                    mm(sr[:], ka_sl, Qa[h][:, qc * 512:(qc + 1) * 512])
                    mm(si[:], ka_sl, Qb[h][:, qc * 512:(qc + 1) * 512])
                    tsl = t[:, qc * 512:(qc + 1) * 512]
                    if (nsq % 4) / 4.0 < act_frac:
                        nc.scalar.activation(tsl, sr[:], AF.Square, scale=0.125)
                    else:
                        nc.vector._custom_dve(sqsc, out=tsl, in0=sr[:], s0=0.125)
                    nsq += 1
                    nc.vector._custom_dve(
                        sqadd,
                        out=u[:, kb * NQ + qc * 512: kb * NQ + (qc + 1) * 512],
                        in0=si[:], in1=tsl, s0=0.125)

        def emit_sqrt(h, pin):
            # one wide sqrt per head amortizes the ~300ns ACT fixed overhead
            u = u_t[h]
            si_ = nc.scalar.activation(u[:], u[:], AF.Sqrt)
            if pin is not None:
                tile.add_dep_helper(si_.ins, pin.ins, sync=False,
                                    reason="act phase order")
            return si_

        def emit_b2(h, pin):
            # exp + attention + denominator + normalized PSUM drain.
            # den[1,q] accumulates via an M=1 ones-matmul; it is then
            # pad-corrected, broadcast across partitions (gpsimd, SBUF-only),
            # reciprocal'd full-rate on the DVE, and folded into the PSUM
            # drain of the attention accumulator (one tensor-multiply
            # instead of a plain cast -- normalization costs nothing extra).
            attn_ps = psum.tile([128, NQ], F32, tag="attn", name="attn_ps")
            den_ps = psum.tile([1, NQ], F32, tag="den", name="den_ps")
            u = u_t[h]
            last = None
            for sl in range(0, KB * NQ, 2 * NQ):
                sw = min(2 * NQ, KB * NQ - sl)
                last = nc.scalar.activation(u[:, sl:sl + sw],
                                            u[:, sl:sl + sw], AF.Exp)
                tile.add_dep_helper(last.ins, pin.ins, sync=False,
                                    reason="act phase order")
            for kb in range(KB):
                for qc in range(2):
                    usl = u[:, kb * NQ + qc * 512: kb * NQ + (qc + 1) * 512]
                    mm(attn_ps[:, qc * 512:(qc + 1) * 512],
                       VK[0:128, kb * 512 + h * 128: kb * 512 + (h + 1) * 128],
                       usl, start=(kb == 0), stop=(kb == KB - 1), skip=True)
                    mm(den_ps[:, qc * 512:(qc + 1) * 512],
                       ones_sb[:, 0:1], usl,
                       start=(kb == 0), stop=(kb == KB - 1), skip=True)
            dsb = const.tile([1, NQ], F32, tag="densb", bufs=2, name="den_sb")
            nc.vector.tensor_scalar_add(dsb[:], den_ps[:], npn_sb[0:1, :])
            rb = const.tile([128, NQ], F32, tag="rb", bufs=2, name="rb")
            nc.gpsimd.partition_broadcast(rb[:], dsb[:], 128)
            nc.vector.reciprocal(rb[:], rb[:])
            att = const.tile([128, NQ], BF16, tag=f"att{h}", name=f"att{h}")
            nc.vector.tensor_mul(att[:, 0:512], attn_ps[:, 0:512],
                                 rb[:, 0:512])
            nc.vector.tensor_mul(att[:, 512:1024], attn_ps[:, 512:1024],
                                 rb[:, 512:1024])
            ATT[h] = att
            return last

        emit_scores(0, 0.75)
        emit_scores(1, 0.75)
        s0 = emit_sqrt(0, None)
        s1 = emit_sqrt(1, None)
        emit_scores(2, 0.25)
        emit_scores(3, 0.25)
        e0 = emit_b2(0, s1)
        e1 = emit_b2(1, s1)
        s2 = emit_sqrt(2, e1)
        s3 = emit_sqrt(3, e1)
        e2 = emit_b2(2, s3)
        e3 = emit_b2(3, s3)

        # ---- output projection: plain PSUM accumulation over heads -------
        for qb in range(8):
            op_ps = ps512("sr0" if qb % 2 == 0 else "sr1")
            for h in range(NHL):
                mm(op_ps[:], ATT[h][:, qb * 128:(qb + 1) * 128], wo_sb[h][:],
                   start=(h == 0), stop=(h == NHL - 1))
            a = outp.tile([128, 512], F32, tag="acc", name="acc")
            if qb % 2 == 0:
                nc.vector.tensor_copy(a[:], op_ps[:])
            else:
                nc.scalar.copy(a[:], op_ps[:])
            nc.sync.dma_start(outq[qb * 128:(qb + 1) * 128, :], a[:])

    nc.compile()
    _BUILD_CACHE[nkp] = nc
    return nc


# ----------------------------------------------------------------------------
# host-side prep / gather
# ----------------------------------------------------------------------------
def _prep_inputs(Q_real, Q_imag, K_real, K_imag, V_real, V_imag,
                 WQ_r, WQ_i, WK_r, WK_i, WV_r, WV_i, WO_r, WO_i, mask):
    f32 = np.float32
    mask = np.asarray(mask).astype(bool)
    cnts = mask.sum(1)
    valid = mask.any(1)
    nkp = int(max(F32MIN_PAD, ((int(cnts.max()) + 127) // 128) * 128)) if valid.any() else F32MIN_PAD

    # weight stacks (shared across cores up to head-group slicing)
    A_q = np.concatenate([WQ_r.T, -WQ_i.T], 0).astype(f32)   # [512, 512]
    B_q = np.concatenate([WQ_i.T, WQ_r.T], 0).astype(f32)
    A_k = np.concatenate([WK_r.T, -WK_i.T], 0).astype(f32)
    B_k = np.concatenate([WK_i.T, WK_r.T], 0).astype(f32)
    A_v = np.concatenate([WV_r.T, -WV_i.T], 0).astype(f32)
    B_v = np.concatenate([WV_i.T, WV_r.T], 0).astype(f32)

    ones1 = np.ones((128, 1), _BF16)

    in_maps = []
    for core in range(NCORES):
        b, hg = core // 2, core % 2
        idx = np.flatnonzero(mask[b])
        cnt = len(idx)

        def cpad(x):  # [Nk, R] -> gathered+padded [nkp, R]
            out = np.zeros((nkp, R), f32)
            out[:cnt] = x[idx]
            return out

        qtf = np.concatenate([Q_real[b].T, Q_imag[b].T], 0).astype(_BF16)    # [512, NQ]
        ktf = np.concatenate([cpad(K_real[b]).T, cpad(K_imag[b]).T], 0).astype(_BF16)
        vtf = np.concatenate([cpad(V_real[b]).T, cpad(V_imag[b]).T], 0).astype(_BF16)

        wq_l = np.empty((NHL, 512, 256), _BF16)
        wk_l = np.empty((NHL, 512, 128), _BF16)
        wv_l = np.empty((512, 512), _BF16)
        wo_l = np.empty((NHL, 128, 512), _BF16)
        for h in range(NHL):
            g = hg * NHL + h
            gc = slice(g * DK, (g + 1) * DK)
            wq_l[h, :, 0:64] = A_q[:, gc]
            wq_l[h, :, 64:128] = B_q[:, gc]
            wq_l[h, :, 128:192] = B_q[:, gc]
            wq_l[h, :, 192:256] = -A_q[:, gc]
            wk_l[h, :, 0:64] = A_k[:, gc]
            wk_l[h, :, 64:128] = B_k[:, gc]
            wv_l[:, h * 128:h * 128 + 64] = A_v[:, gc]
            wv_l[:, h * 128 + 64:(h + 1) * 128] = B_v[:, gc]
            # q-orientation outproj: out[q, 0:256]=y_r, out[q, 256:512]=y_i
            # rows 0:64 = attn real dims, 64:128 = attn imag dims
            wo_l[h, 0:64, 0:256] = WO_r[:, gc].T
            wo_l[h, 64:128, 0:256] = -WO_i[:, gc].T
            wo_l[h, 0:64, 256:512] = WO_i[:, gc].T
            wo_l[h, 64:128, 256:512] = WO_r[:, gc].T

        npn_ = np.full((128, 1), -(nkp - cnt), f32)
        in_maps.append({
            "qt": qtf, "kt": ktf, "vt": vtf,
            "wq": wq_l, "wk": wk_l, "wv": wv_l, "wo": wo_l,
            "ones": ones1, "npn": npn_,
        })
    return in_maps, nkp, valid


def _gather(results, valid):
    out = np.zeros((B, NQ, R), np.complex64)
    for b in range(B):
        if not valid[b]:
            continue
        o = results[2 * b]["outq"] + results[2 * b + 1]["outq"]   # [NQ, 512]
        out[b] = o[:, 0:256] + 1j * o[:, 256:512]
    return out


def _run(inputs, trace=False, trace_kwargs=None):
    from concourse.bass_utils import run_bass_kernel_spmd
    in_maps, nkp, valid = _prep_inputs(**inputs)
    nc = _build(nkp)
    res = run_bass_kernel_spmd(nc, in_maps, core_ids=list(range(NCORES)),
                               trace=trace, **(trace_kwargs or {}))
    return _gather(res.results, valid), res


def kernel(**inputs) -> np.ndarray:
    out, _ = _run(inputs)
    return out


# revision 27
# speedup vs baseline: 1.1736x; 1.1736x over previous
"""Trainium2 Bass kernel for complex-valued multi-head attention with key masking.

Problem (hardcoded shapes): B=4, Nq=Nk=1024, R=256, NH=8, DK=DV=64.
  Q,K,V complex [B,N,R] (given as _real/_imag f32 pairs), complex weights
  WQ/WK/WV [512,256], WO [256,512], boolean key mask [B,Nk].
  out = complex MHA(Q,K,V) with softmax over |scores| restricted to valid keys.

Sharding: 8 cores = (batch b in 0..3) x (head-group hg in 0..1, 4 heads each).
Each core computes its batch's attention for its 4 heads plus the partial
output projection; the host sums the two head-group partials per batch.

Device-side layout: channels on partitions, sequence on the free dim, with
complex arithmetic folded into the matmuls by stacking real/imag along the
128-partition contraction dim (Qa=[Qp_r;Qp_i], Qb=[Qp_i;-Qp_r], Ka=[Kp_r;Kp_i]
give Sr/Si with one full-width matmul each).

Key structural idea vs a [q,k]-softmax design: scores are computed
TRANSPOSED, S^T[k,q] = Ka_blk^T.Qa per 128-key block, so the whole softmax
chain (|s|^2 via one fused two-source DVE op, sqrt, exp) runs in [k,q]
layout and the exp output E^T is consumed by the attention matmul straight
from SBUF -- no transpose DMA, no DRAM bounce.  The softmax denominator
den[h,q] = sum_k E^T is produced on the PE by a ones-column matmul
accumulated into a [4,1024] PSUM tile, moved to [q,4-heads] orientation by a
tiny identity matmul, reciprocal'd once, and the per-head 1/den is folded
into the output-projection reduction (out_q[q,r] = sum_h (att_h^T.WO_h)/den_h)
which runs in q-partition orientation so the scale is a per-partition scalar.
Masked keys are removed by host-side compaction (padded keys contribute
exp(0)=1 to den, subtracted via a host-provided count).
"""

import numpy as np
import ml_dtypes

B, NQ, NK, R = 4, 1024, 1024, 256
NH, DK, DV = 8, 64, 64
NCORES = 8
NHL = 4          # heads per core
F32MIN_PAD = 640  # minimum padded key count (keys padded to a multiple of 128)

_BF16 = ml_dtypes.bfloat16

# ----------------------------------------------------------------------------
# custom DVE op (registered at import into concourse's op table)
# ----------------------------------------------------------------------------
_OPS = {}


def _register_custom_ops():
    if _OPS:
        return
    import concourse.dve_ops as dom
    from concourse.dve_ops import DveOp
    from concourse.dve_spec import Spec, Src0, Src1, C0, sq, lower, _has_src1
    from concourse.dve_uop import DveOpSpec

    def make(name, spec):
        if name in dom._SUB_OPCODE_FOR_NAME:
            _OPS[name] = next(o for o in dom.OPS if o.name == name)
            return
        row = dom._CUSTOM_DVE_ROW_BASE + len(dom.OPS)
        assert row < 0x20, "custom DVE row overflow"
        shas = {}
        for ver in ("v3", "v4"):
            tmp = DveOpSpec(name=name, opcode=row, uops=lower(spec, ver=ver),
                            rd1_en=_has_src1(spec))
            shas[ver] = tmp.sha(ver)
        op = DveOp(name, spec, subdim=False, uops_sha=shas)
        dom.OPS.append(op)
        dom._SUB_OPCODE_FOR_NAME[name] = row
        dom.CUSTOM_DVE_SPECS[name] = spec
        _OPS[name] = op

    # t = (in0*s0)^2          (drains+squares one score tile from PSUM)
    make("CMHA_SQSC", Spec(
        body=sq(Src0 * C0),
        reference=lambda in0, in1, s0, s1, imm2: (in0.astype(np.float32) * s0) ** 2,
    ))
    # u = (in0*s0)^2 + in1    (second square + accumulate |s/8|^2; the DVE
    # can read at most one input from PSUM, so this pairs with CMHA_SQSC)
    make("CMHA_SQADD", Spec(
        body=sq(Src0 * C0) + Src1,
        reference=lambda in0, in1, s0, s1, imm2: (in0.astype(np.float32) * s0) ** 2
        + in1.astype(np.float32),
    ))


# ----------------------------------------------------------------------------
# device program
# ----------------------------------------------------------------------------
_BUILD_CACHE = {}


def _build(nkp):
    """Build + compile the SPMD device program for padded key count nkp."""
    if nkp in _BUILD_CACHE:
        return _BUILD_CACHE[nkp]
    _register_custom_ops()
    import concourse.bass as bass
    import concourse.bacc as bacc
    import concourse.mybir as mybir
    import concourse.tile as tile
    from contextlib import ExitStack

    F32 = mybir.dt.float32
    BF16 = mybir.dt.bfloat16
    AF = mybir.ActivationFunctionType
    assert nkp % 128 == 0
    KB = nkp // 128                  # 128-sized key blocks, all full

    nc = bacc.Bacc("TRN2", target_bir_lowering=False, debug=False,
                   num_devices=NCORES)

    qt = nc.dram_tensor("qt", [512, NQ], BF16, kind="ExternalInput").ap()
    kt = nc.dram_tensor("kt", [512, nkp], BF16, kind="ExternalInput").ap()
    vt = nc.dram_tensor("vt", [512, nkp], BF16, kind="ExternalInput").ap()
    wq = nc.dram_tensor("wq", [NHL, 512, 256], BF16, kind="ExternalInput").ap()
    wk = nc.dram_tensor("wk", [NHL, 512, 128], BF16, kind="ExternalInput").ap()
    wv = nc.dram_tensor("wv", [512, 512], BF16, kind="ExternalInput").ap()
    wo = nc.dram_tensor("wo", [NHL, 128, 512], BF16, kind="ExternalInput").ap()
    ones = nc.dram_tensor("ones", [128, 1], BF16, kind="ExternalInput").ap()
    npn = nc.dram_tensor("npn", [128, 1], F32, kind="ExternalInput").ap()
    outq = nc.dram_tensor("outq", [NQ, 512], F32, kind="ExternalOutput").ap()

    sqsc = _OPS["CMHA_SQSC"]
    sqadd = _OPS["CMHA_SQADD"]
    ADD = mybir.AluOpType.add
    MULT = mybir.AluOpType.mult

    with tile.TileContext(nc) as tc, ExitStack() as ctx:
        const = ctx.enter_context(tc.tile_pool(name="const", bufs=1))
        psum = ctx.enter_context(tc.tile_pool(name="psum", bufs=1, space="PSUM"))
        outp = ctx.enter_context(tc.tile_pool(name="outp", bufs=4))

        # ---- input loads: all on the hardware-DGE engines (sync/scalar);
        # gpsimd software descriptor-gen costs ~0.8us per DMA and delays
        # the first matmul.  Queue order matches consumption order.
        def load(shape, dtype, src, tag, eng):
            t = const.tile(shape, dtype, tag=tag, name=tag)
            eng.dma_start(t[:], src)
            return t

        qt_sb = [load([128, NQ], BF16, qt[c * 128:(c + 1) * 128, :], f"qt{c}",
                      nc.sync) for c in range(4)]
        # wq[h] is [512, 256] = 4 contraction chunks; load per-head in one DMA
        wq_t = []
        for h in range(NHL):
            t = const.tile([128, 1024], BF16, tag=f"wq{h}", name=f"wqt{h}")
            nc.sync.dma_start(
                t[:].rearrange("p (c n) -> p c n", c=4),
                wq[h].rearrange("(c p) n -> p c n", p=128))
            wq_t.append(t)
        wq_sb = [[wq_t[h][:, c * 256:(c + 1) * 256] for c in range(4)]
                 for h in range(NHL)]
        kt_sb = [load([128, nkp], BF16, kt[c * 128:(c + 1) * 128, :], f"kt{c}",
                      nc.scalar) for c in range(4)]
        wk_t = []
        for h in range(NHL):
            t = const.tile([128, 512], BF16, tag=f"wk{h}", name=f"wkt{h}")
            nc.scalar.dma_start(
                t[:].rearrange("p (c n) -> p c n", c=4),
                wk[h].rearrange("(c p) n -> p c n", p=128))
            wk_t.append(t)
        wk_sb = [[wk_t[h][:, c * 128:(c + 1) * 128] for c in range(4)]
                 for h in range(NHL)]
        vt_sb = [load([128, nkp], BF16, vt[c * 128:(c + 1) * 128, :], f"vt{c}",
                      nc.sync) for c in range(4)]
        wv_sb = [load([128, 512], BF16, wv[c * 128:(c + 1) * 128, :], f"wv{c}",
                      nc.scalar) for c in range(4)]
        wo_sb = [load([128, 512], BF16, wo[h], f"wo{h}", nc.scalar)
                 for h in range(NHL)]
        ones_sb = load([128, 1], BF16, ones[:], "ones", nc.sync)
        npn_sb = load([128, 1], F32, npn[:], "npn", nc.sync)

        VK = const.tile([128, 512 * KB], BF16, tag="vk", name="VK")

        # PSUM tags: 4x [128,512] (sr0/sr1/si0/si1 rings, reused by the
        # projections, the den transpose and the outproj partials), one
        # [128,1024] attn accumulator, one [4,1024] den accumulator = 8 banks.
        def ps512(tag):
            return psum.tile([128, 512], F32, tag=tag, name=tag)

        def mm(out_ap, lhsT, rhs, start=True, stop=True, skip=False):
            nc.tensor.matmul(out_ap, lhsT, rhs, start=start, stop=stop,
                             skip_group_check=skip)

        # ---- phase A: projections --------------------------------------
        # copies alternate vector/scalar (gpsimd cannot read PSUM on trn2)
        _cp_i = [0]

        def copy(dst, src):
            _cp_i[0] += 1
            if _cp_i[0] % 2:
                nc.vector.tensor_copy(dst, src)
            else:
                nc.scalar.copy(dst, src)

        Qa, Qb, Ka = [], [], []
        for h in range(NHL):
            qa = const.tile([128, NQ], BF16, tag=f"qa{h}", name=f"Qa{h}")
            qb = const.tile([128, NQ], BF16, tag=f"qb{h}", name=f"Qb{h}")
            for qc in range(2):
                pa = ps512("sr0" if qc == 0 else "sr1")
                pb = ps512("si0" if qc == 0 else "si1")
                for c in range(4):
                    mm(pa[:], wq_sb[h][c][:, 0:128],
                       qt_sb[c][:, qc * 512:(qc + 1) * 512], c == 0, c == 3)
                for c in range(4):
                    mm(pb[:], wq_sb[h][c][:, 128:256],
                       qt_sb[c][:, qc * 512:(qc + 1) * 512], c == 0, c == 3)
                copy(qa[:, qc * 512:(qc + 1) * 512], pa[:])
                copy(qb[:, qc * 512:(qc + 1) * 512], pb[:])
            Qa.append(qa)
            Qb.append(qb)

            ka = const.tile([128, nkp], BF16, tag=f"ka{h}", name=f"Ka{h}")
            for o in range(0, nkp, 512):
                w_ = min(512, nkp - o)
                pk = ps512("sr0" if (o // 512) % 2 == 0 else "sr1")
                for c in range(4):
                    mm(pk[0:128, 0:w_], wk_sb[h][c][:],
                       kt_sb[c][:, o:o + w_], c == 0, c == 3)
                copy(ka[:, o:o + w_], pk[0:128, 0:w_])
            Ka.append(ka)

        for kb in range(KB):
            pv = ps512("si0" if kb % 2 == 0 else "si1")
            for c in range(4):
                mm(pv[:], vt_sb[c][:, kb * 128:(kb + 1) * 128],
                   wv_sb[c][:], c == 0, c == 3)
            copy(VK[0:128, kb * 512:(kb + 1) * 512], pv[:])

        # ---- phase B: head-pair pipeline ---------------------------------
        # DVE can read only ONE PSUM input per op, so |s|^2 is a two-op
        # chain: t = (Sr/8)^2 (SQSC on DVE, or Square on ACT -- Square is
        # in every ACT table), u = (Si/8)^2 + t (SQADD, DVE).  Heads run in
        # pairs: {scores 01} {sqrt 01} {scores 23, exp 01 + attn/den 01}
        # {sqrt 23} {exp 23 + attn/den 23}, so the PE stays dense while the
        # ACT table only switches 4x and attention streams behind exp.
        u_t = [const.tile([128, KB * NQ], BF16, tag=f"u{h}", name=f"u{h}")
               for h in range(NHL)]
        ATT = [None] * NHL

        def emit_scores(h, act_frac):
            # act_frac of the first-squares go to ACT, rest to DVE
            u = u_t[h]
            nsq = 0
            for kb in range(KB):
                ka_sl = Ka[h][:, kb * 128:(kb + 1) * 128]
                t = const.tile([128, NQ], BF16, tag="tsq", bufs=3, name="tsq")
                for qc in range(2):
                    sr = ps512("sr0" if qc == 0 else "sr1")
                    si = ps512("si0" if qc == 0 else "si1")
                    mm(sr[:], ka_sl, Qa[h][:, qc * 512:(qc + 1) * 512])
                    mm(si[:], ka_sl, Qb[h][:, qc * 512:(qc + 1) * 512])
                    tsl = t[:, qc * 512:(qc + 1) * 512]
                    if (nsq % 4) / 4.0 < act_frac:
                        nc.scalar.activation(tsl, sr[:], AF.Square, scale=0.125)
                    else:
                        nc.vector._custom_dve(sqsc, out=tsl, in0=sr[:], s0=0.125)
                    nsq += 1
                    nc.vector._custom_dve(
                        sqadd,
                        out=u[:, kb * NQ + qc * 512: kb * NQ + (qc + 1) * 512],
                        in0=si[:], in1=tsl, s0=0.125)

        def emit_sqrt(h, pin):
            # one wide sqrt per head amortizes the ~300ns ACT fixed overhead
            u = u_t[h]
            si_ = nc.scalar.activation(u[:], u[:], AF.Sqrt)
            if pin is not None:
                tile.add_dep_helper(si_.ins, pin.ins, sync=False,
                                    reason="act phase order")
            return si_

        def emit_b2(h, pin):
            # exp + attention + denominator + normalized PSUM drain.
            # den[1,q] accumulates via an M=1 ones-matmul; it is then
            # pad-corrected, broadcast across partitions (gpsimd, SBUF-only),
            # reciprocal'd full-rate on the DVE, and folded into the PSUM
            # drain of the attention accumulator (one tensor-multiply
            # instead of a plain cast -- normalization costs nothing extra).
            attn_ps = psum.tile([128, NQ], F32, tag="attn", name="attn_ps")
            den_ps = psum.tile([1, NQ], F32, tag="den", name="den_ps")
            u = u_t[h]
            last = None
            for sl in range(0, KB * NQ, 2 * NQ):
                sw = min(2 * NQ, KB * NQ - sl)
                last = nc.scalar.activation(u[:, sl:sl + sw],
                                            u[:, sl:sl + sw], AF.Exp)
                tile.add_dep_helper(last.ins, pin.ins, sync=False,
                                    reason="act phase order")
            for kb in range(KB):
                for qc in range(2):
                    usl = u[:, kb * NQ + qc * 512: kb * NQ + (qc + 1) * 512]
                    mm(attn_ps[:, qc * 512:(qc + 1) * 512],
                       VK[0:128, kb * 512 + h * 128: kb * 512 + (h + 1) * 128],
                       usl, start=(kb == 0), stop=(kb == KB - 1), skip=True)
                    mm(den_ps[:, qc * 512:(qc + 1) * 512],
                       ones_sb[:, 0:1], usl,
                       start=(kb == 0), stop=(kb == KB - 1), skip=True)
            dsb = const.tile([1, NQ], F32, tag="densb", bufs=2, name="den_sb")
            nc.vector.tensor_scalar_add(dsb[:], den_ps[:], npn_sb[0:1, :])
            rb = const.tile([128, NQ], F32, tag="rb", bufs=2, name="rb")
            nc.gpsimd.partition_broadcast(rb[:], dsb[:], 128)
            nc.vector.reciprocal_approx_fast(out=rb[:], in_=rb[:])
            att = const.tile([128, NQ], BF16, tag=f"att{h}", name=f"att{h}")
            nc.vector.tensor_mul(att[:, 0:512], attn_ps[:, 0:512],
                                 rb[:, 0:512])
            nc.vector.tensor_mul(att[:, 512:1024], attn_ps[:, 512:1024],
                                 rb[:, 512:1024])
            ATT[h] = att
            return last

        emit_scores(0, 0.75)
        emit_scores(1, 0.75)
        s0 = emit_sqrt(0, None)
        s1 = emit_sqrt(1, None)
        emit_scores(2, 0.25)
        emit_scores(3, 0.25)
        e0 = emit_b2(0, s1)
        e1 = emit_b2(1, s1)
        s2 = emit_sqrt(2, e1)
        s3 = emit_sqrt(3, e1)
        e2 = emit_b2(2, s3)
        e3 = emit_b2(3, s3)

        # ---- output projection: plain PSUM accumulation over heads -------
        for qb in range(8):
            op_ps = ps512("sr0" if qb % 2 == 0 else "sr1")
            for h in range(NHL):
                mm(op_ps[:], ATT[h][:, qb * 128:(qb + 1) * 128], wo_sb[h][:],
                   start=(h == 0), stop=(h == NHL - 1))
            a = outp.tile([128, 512], F32, tag="acc", name="acc")
            if qb % 2 == 0:
                nc.vector.tensor_copy(a[:], op_ps[:])
            else:
                nc.scalar.copy(a[:], op_ps[:])
            nc.sync.dma_start(outq[qb * 128:(qb + 1) * 128, :], a[:])

    nc.compile()
    _BUILD_CACHE[nkp] = nc
    return nc


# ----------------------------------------------------------------------------
# host-side prep / gather
# ----------------------------------------------------------------------------
def _prep_inputs(Q_real, Q_imag, K_real, K_imag, V_real, V_imag,
                 WQ_r, WQ_i, WK_r, WK_i, WV_r, WV_i, WO_r, WO_i, mask):
    f32 = np.float32
    mask = np.asarray(mask).astype(bool)
    cnts = mask.sum(1)
    valid = mask.any(1)
    nkp = int(max(F32MIN_PAD, ((int(cnts.max()) + 127) // 128) * 128)) if valid.any() else F32MIN_PAD

    # weight stacks (shared across cores up to head-group slicing)
    A_q = np.concatenate([WQ_r.T, -WQ_i.T], 0).astype(f32)   # [512, 512]
    B_q = np.concatenate([WQ_i.T, WQ_r.T], 0).astype(f32)
    A_k = np.concatenate([WK_r.T, -WK_i.T], 0).astype(f32)
    B_k = np.concatenate([WK_i.T, WK_r.T], 0).astype(f32)
    A_v = np.concatenate([WV_r.T, -WV_i.T], 0).astype(f32)
    B_v = np.concatenate([WV_i.T, WV_r.T], 0).astype(f32)

    ones1 = np.ones((128, 1), _BF16)

    in_maps = []
    for core in range(NCORES):
        b, hg = core // 2, core % 2
        idx = np.flatnonzero(mask[b])
        cnt = len(idx)

        def cpad(x):  # [Nk, R] -> gathered+padded [nkp, R]
            out = np.zeros((nkp, R), f32)
            out[:cnt] = x[idx]
            return out

        qtf = np.concatenate([Q_real[b].T, Q_imag[b].T], 0).astype(_BF16)    # [512, NQ]
        ktf = np.concatenate([cpad(K_real[b]).T, cpad(K_imag[b]).T], 0).astype(_BF16)
        vtf = np.concatenate([cpad(V_real[b]).T, cpad(V_imag[b]).T], 0).astype(_BF16)

        wq_l = np.empty((NHL, 512, 256), _BF16)
        wk_l = np.empty((NHL, 512, 128), _BF16)
        wv_l = np.empty((512, 512), _BF16)
        wo_l = np.empty((NHL, 128, 512), _BF16)
        for h in range(NHL):
            g = hg * NHL + h
            gc = slice(g * DK, (g + 1) * DK)
            wq_l[h, :, 0:64] = A_q[:, gc]
            wq_l[h, :, 64:128] = B_q[:, gc]
            wq_l[h, :, 128:192] = B_q[:, gc]
            wq_l[h, :, 192:256] = -A_q[:, gc]
            wk_l[h, :, 0:64] = A_k[:, gc]
            wk_l[h, :, 64:128] = B_k[:, gc]
            wv_l[:, h * 128:h * 128 + 64] = A_v[:, gc]
            wv_l[:, h * 128 + 64:(h + 1) * 128] = B_v[:, gc]
            # q-orientation outproj: out[q, 0:256]=y_r, out[q, 256:512]=y_i
            # rows 0:64 = attn real dims, 64:128 = attn imag dims
            wo_l[h, 0:64, 0:256] = WO_r[:, gc].T
            wo_l[h, 64:128, 0:256] = -WO_i[:, gc].T
            wo_l[h, 0:64, 256:512] = WO_i[:, gc].T
            wo_l[h, 64:128, 256:512] = WO_r[:, gc].T

        npn_ = np.full((128, 1), -(nkp - cnt), f32)
        in_maps.append({
            "qt": qtf, "kt": ktf, "vt": vtf,
            "wq": wq_l, "wk": wk_l, "wv": wv_l, "wo": wo_l,
            "ones": ones1, "npn": npn_,
        })
    return in_maps, nkp, valid


def _gather(results, valid):
    out = np.zeros((B, NQ, R), np.complex64)
    for b in range(B):
        if not valid[b]:
            continue
        o = results[2 * b]["outq"] + results[2 * b + 1]["outq"]   # [NQ, 512]
        out[b] = o[:, 0:256] + 1j * o[:, 256:512]
    return out


def _run(inputs, trace=False, trace_kwargs=None):
    from concourse.bass_utils import run_bass_kernel_spmd
    in_maps, nkp, valid = _prep_inputs(**inputs)
    nc = _build(nkp)
    res = run_bass_kernel_spmd(nc, in_maps, core_ids=list(range(NCORES)),
                               trace=trace, **(trace_kwargs or {}))
    return _gather(res.results, valid), res


def kernel(**inputs) -> np.ndarray:
    out, _ = _run(inputs)
    return out
